# revision 7
# baseline (speedup 1.0000x reference)
"""Trainium2 Bass kernel for the DEN (Mahalanobis distance) layer.

Computes out[b, e] = (x_b - c_e)^T Sigma_e^{-1} (x_b - c_e) for
x [8192, 128], Centroids [128, 1, 128], Sigmas [128, 128, 128].

Strategy (v3)
-------------
Wrapped-diagonal decomposition (see module history): 66 coefficient packs
(linear, x^2, 64 off-diagonal products) feed a PSUM-accumulated chain of
[128,128]x[128,512] bf16 matmuls over two PSUM banks.  Probing showed
N=512 bf16 matmuls cost ~430 ns each regardless of dependencies or weight
reuse, so the 132-matmul chain (~55 us) IS the PE floor; v3 therefore
attacks everything around it:

1. PE warmup: 7 dummy matmuls on uninitialized SBUF issue at t=0 (no DMA
   dependency) so the HAM clock gate's ~3.4 us busy window elapses during
   the input DMA and the real chain runs at the warm clock from its first
   instruction.
2. Head: the first coefficient chunk carries only the linear + x^2 packs
   (64 KB), so the chain starts after ~320 KB of DMA instead of ~1 MB.
3. Product work split: DVE keeps 56 packs (~38 us, safely under the PE
   floor), GpSimd takes 8 (G3 early, G10b late), scalar engine does x^2.
   Single accumulation chain => slow-engine packs are ordered last.
4. Tail: per-bank eviction (Act bank0 / DVE bank1) with per-bank output
   DMAs, overlapping the final matmuls.

Sharding: data-parallel over batch B across the 8 cores (1024 rows each);
coefficient packs (derived from Sigmas/Centroids) are replicated.
"""

import os
import sys

sys.path.insert(0, "/opt/trn_rl_repo")

import numpy as np
import ml_dtypes

E, B, D = 128, 8192, 128
NCORES = 8
BLOC = B // NCORES          # 1024 batch rows per core
BT = 512                    # matmul free-dim tile (one PSUM bank)
NSLOT = 16                  # rotation slots: 0..7 then 8,16,...,64
ROTVALS = (1, 2, 3, 4, 5, 6, 7) + tuple(range(8, 65, 8))
NWARM = 7                   # dummy matmuls to trip the HAM clock gate


def _slotval(s):
    return s if s <= 7 else 8 * (s - 7)


# product groups: (name, in0 slot range [lo,hi), in1 slot, engine).
# Column i of a group is the product rot[lo+i] * rot[in1], covering
# diagonal j = slotval(in1) - slotval(lo+i) with row rotation a = lo+i.
GROUPS = [
    ("g1", 0, 2, 2, "dve"),     # j = 2,1
    ("g2", 0, 2, 4, "dve"),     # j = 4,3
    ("g3", 0, 4, 8, "gp"),      # j = 8..5   (GpSimd, early)
    ("g4", 0, 8, 9, "dve"),     # j = 16..9
    ("g5", 0, 8, 10, "dve"),    # j = 24..17
    ("g6", 0, 8, 11, "dve"),    # j = 32..25
    ("g7", 0, 8, 12, "dve"),    # j = 40..33
    ("g8", 0, 8, 13, "dve"),    # j = 48..41
    ("g9", 0, 8, 14, "dve"),    # j = 56..49
    ("g10a", 0, 4, 15, "dve"),  # j = 64..61
    ("g10b", 4, 8, 15, "gp"),   # j = 60..57 (GpSimd, late)
]

# matmul chain order: linear, x^2, DVE groups, then GpSimd groups (their
# products land late; chain position within one PSUM chain is free).
CHAIN = ["lin", "p0"] + [g[0] for g in GROUPS if g[4] == "dve"] \
        + [g[0] for g in GROUPS if g[4] == "gp"]

_GBY = {g[0]: g for g in GROUPS}
ORDER = []   # diagonal j per quad-pack position (positions 2..65)
AVAL = []    # row rotation a per quad-pack position
for _name in CHAIN[2:]:
    _, _lo, _hi, _s1, _ = _GBY[_name]
    for _i in range(_hi - _lo):
        ORDER.append(_slotval(_s1) - _slotval(_lo + _i))
        AVAL.append(_lo + _i)

NPACK = 2 + len(ORDER)      # 66 emission positions
# coefficient chunks: c0 = {linear, x^2} (tiny, unblocks the chain early);
# then 8 chunks of 8 packs
CHUNKS = [2] + [8] * 8
CHUNK_OFF = [0]
for _n in CHUNKS:
    CHUNK_OFF.append(CHUNK_OFF[-1] + _n)


def _chunk_of(pos):
    for ci, off in enumerate(CHUNK_OFF[1:]):
        if pos < off:
            return ci, pos - CHUNK_OFF[ci]
    raise ValueError(pos)


bf16 = ml_dtypes.bfloat16

_STATE: dict = {}


def _build_module():
    import concourse.bacc as bacc
    import concourse.tile as tile
    import concourse.mybir as mybir
    from contextlib import ExitStack

    nc = bacc.Bacc("TRN2", target_bir_lowering=False, debug=False)

    xr_d = nc.dram_tensor("xrot", [NSLOT, D, BLOC], mybir.dt.bfloat16,
                          kind="ExternalInput")
    cw_d = nc.dram_tensor("cw", [D, NPACK * E], mybir.dt.bfloat16, kind="ExternalInput")
    tv_d = nc.dram_tensor("tv", [E, 1], mybir.dt.float32, kind="ExternalInput")
    out_d = nc.dram_tensor("out", [E, BLOC], mybir.dt.float32, kind="ExternalOutput")

    f32 = mybir.dt.float32
    b16 = mybir.dt.bfloat16
    Ident = mybir.ActivationFunctionType.Identity

    with tile.TileContext(nc) as tc, ExitStack() as ctx:
        const_pool = ctx.enter_context(tc.tile_pool(name="const", bufs=1))
        coef_pool = ctx.enter_context(tc.tile_pool(name="coef", bufs=1))
        p0_pool = ctx.enter_context(tc.tile_pool(name="p0", bufs=1))
        g_pool = ctx.enter_context(tc.tile_pool(name="g", bufs=3))
        gp_pool = ctx.enter_context(tc.tile_pool(name="gp", bufs=2))
        psum_pool = ctx.enter_context(tc.tile_pool(name="acc", bufs=2, space="PSUM"))
        out_pool = ctx.enter_context(tc.tile_pool(name="outs", bufs=2))

        # PE warmup: dummy matmuls on an uninitialized tile, no DMA deps.
        # They issue at t=0 and keep the PE busy past the HAM activity
        # window so the real chain runs at the warm clock.
        WU = const_pool.tile([D, BT], b16, tag="warm")
        nc.gpsimd.memset(WU[:, :], 0)
        PSW = psum_pool.tile([E, BT], f32, tag="psw", name="psw")
        for _ in range(NWARM):
            nc.tensor.matmul(PSW[:, :], WU[:, 0:E], WU[:, :],
                             start=True, stop=True, skip_group_check=True)

        ROTS = const_pool.tile([D, NSLOT * BLOC], b16, tag="rots")
        R3 = ROTS[:, :].rearrange("p (s b) -> p s b", s=NSLOT)
        TV = const_pool.tile([E, 1], f32, tag="tv")
        coef_tiles = [coef_pool.tile([D, n * E], b16, name=f"cw{ci}", tag=f"cw{ci}")
                      for ci, n in enumerate(CHUNKS)]

        def dma_rot(s):
            nc.sync.dma_start(ROTS[:, s * BLOC:(s + 1) * BLOC], xr_d.ap()[s])

        def dma_coef(ci):
            o0, o1 = CHUNK_OFF[ci] * E, CHUNK_OFF[ci + 1] * E
            nc.sync.dma_start(coef_tiles[ci][:], cw_d.ap()[:, o0:o1])

        # DMA emission in consumption order: slot0 + tiny chunk0 unlock the
        # linear and x^2 matmuls; rotations ordered by first product use.
        dma_rot(0)
        dma_coef(0)
        plan = [1, 2, 4, 3, 8, 5, 6, 7, "c1", 9, "c2", 10, "c3", 11, "c4",
                12, "c5", 13, "c6", 14, "c7", 15, "c8"]
        for item in plan:
            if isinstance(item, str):
                dma_coef(int(item[1:]))
            else:
                dma_rot(item)
        nc.sync.dma_start(TV[:], tv_d.ap())

        PS = psum_pool.tile([E, BLOC], f32, tag="ps", name="ps")

        def emit_pack(pos, rhs_ap_fn, start=False, stop=False):
            ci, cc = _chunk_of(pos)
            for bt in range(2):
                nc.tensor.matmul(PS[:, bt * BT:(bt + 1) * BT],
                                 coef_tiles[ci][:, cc * E:(cc + 1) * E],
                                 rhs_ap_fn(bt),
                                 start=start, stop=stop and bt == 1)

        # pos 0: linear term (rhs = x itself), starts the chain
        emit_pack(0, lambda bt: ROTS[:, bt * BT:(bt + 1) * BT], start=True)

        # pos 1: x^2 on the scalar engine
        PK0 = p0_pool.tile([D, BLOC], b16)
        nc.scalar.square(PK0[:, :], ROTS[:, 0:BLOC])
        emit_pack(1, lambda bt: PK0[:, bt * BT:(bt + 1) * BT])

        # product groups on their engines (in production order)
        prod_tiles = {}
        for name, lo, hi, s1, eng in GROUPS:
            w = hi - lo
            pool = g_pool if eng == "dve" else gp_pool
            PKg = pool.tile([D, w * BLOC], b16, name=f"PK{name}",
                            tag=("g" if eng == "dve" else "gp"))
            engine = nc.vector if eng == "dve" else nc.gpsimd
            engine.tensor_mul(
                PKg[:, :].rearrange("p (s b) -> p s b", s=w),
                R3[:, lo:hi, :],
                R3[:, s1:s1 + 1, :].broadcast_to((D, w, BLOC)),
            )
            prod_tiles[name] = PKg

        pos = 2
        for name in CHAIN[2:]:
            _, lo, hi, _, _ = _GBY[name]
            PKg = prod_tiles[name]
            for i in range(hi - lo):
                emit_pack(pos, lambda bt, _i=i: PKg[:, _i * BLOC + bt * BT:
                                                    _i * BLOC + (bt + 1) * BT],
                          stop=(pos == NPACK - 1))
                pos += 1

        # per-bank eviction + output DMA, overlapping the tail matmuls
        OT0 = out_pool.tile([E, BT], f32, tag="ot")
        OT1 = out_pool.tile([E, BT], f32, tag="ot")
        nc.scalar.activation(OT0[:, :], PS[:, 0:BT], Ident, bias=TV[:, 0:1])
        nc.sync.dma_start(out_d.ap()[:, 0:BT], OT0[:, :])
        nc.vector.tensor_scalar_add(OT1[:, :], PS[:, BT:BLOC], TV[:, 0:1])
        nc.sync.dma_start(out_d.ap()[:, BT:BLOC], OT1[:, :])

    nc.compile()
    return nc


def _host_precompute(Centroids: np.ndarray, Sigmas: np.ndarray):
    """Coefficient packs from the (replicated) small parameters."""
    Sinv = np.linalg.inv(Sigmas.astype(np.float64))
    A = 0.5 * (Sinv + np.swapaxes(Sinv, 1, 2))          # [E, D, D] symmetric
    c = Centroids[:, 0, :].astype(np.float64)           # [E, D]
    Ac = np.einsum("edk,ek->ed", A, c)

    cw = np.zeros((D, NPACK, E), np.float32)            # [row, emission pos, e]
    idx = np.arange(D)
    cw[:, 0, :] = (-2.0 * Ac.T)                         # linear term
    cw[:, 1, :] = A[:, idx, idx].T                      # x^2 diagonal
    for k, j in enumerate(ORDER):
        s = 2.0 if 1 <= j <= 63 else 1.0
        a = AVAL[k]
        cw[:, 2 + k, :] = s * A[:, (idx + a) % D, (idx + a + j) % D].T
    cw_host = np.ascontiguousarray(cw.reshape(D, NPACK * E)).astype(bf16)
    tv_host = np.ascontiguousarray(
        np.einsum("ed,ed->e", Ac, c).astype(np.float32)[:, None]
    )                                                                    # [E, 1]
    return cw_host, tv_host


def _get_nc():
    if "nc" not in _STATE:
        os.environ.setdefault("JAX_COMPILATION_CACHE_DIR", "/root/.jax_cache")
        _STATE["nc"] = _build_module()
    return _STATE["nc"]


def _make_in_maps(x, Centroids, Sigmas):
    cw_host, tv_host = _host_precompute(
        np.asarray(Centroids, np.float32), np.asarray(Sigmas, np.float32)
    )
    xT = np.ascontiguousarray(np.asarray(x, np.float32).T).astype(bf16)  # [D, B]
    in_maps = []
    for cidx in range(NCORES):
        xTs = np.ascontiguousarray(xT[:, cidx * BLOC:(cidx + 1) * BLOC])
        xrot = np.stack([xTs] + [np.roll(xTs, -r, axis=0) for r in ROTVALS])
        in_maps.append({
            "xrot": np.ascontiguousarray(xrot),
            "cw": cw_host,
            "tv": tv_host,
        })
    return in_maps


def _run_device(in_maps, trace=False):
    from concourse import bass_utils

    nc = _get_nc()
    return bass_utils.run_bass_kernel_spmd(
        nc, in_maps, core_ids=list(range(NCORES)), trace=trace
    )


def kernel(x, Centroids, Sigmas):
    in_maps = _make_in_maps(x, Centroids, Sigmas)
    res = _run_device(in_maps)
    outT = np.concatenate([res.results[c]["out"] for c in range(NCORES)], axis=1)
    return np.ascontiguousarray(outT.T).astype(np.float32)


# revision 8
# speedup vs baseline: 1.1975x; 1.1975x over previous
"""Trainium2 Bass kernel for the DEN (Mahalanobis distance) layer.

Computes out[b, e] = (x_b - c_e)^T Sigma_e^{-1} (x_b - c_e) for
x [8192, 128], Centroids [128, 1, 128], Sigmas [128, 128, 128].

Strategy (v4)
-------------
Wrapped-diagonal decomposition: 66 coefficient packs (linear, x^2, 64
off-diagonal products) feed a PSUM-accumulated chain of
[128,128]x[128,512] bf16 matmuls over two PSUM banks.  Probing showed
N=512 bf16 matmuls cost ~385-430 ns each regardless of dependencies, so
the 132-matmul chain (~51 us) IS the PE floor; everything else is
arranged to hide under it:

- PE warmup: dummy matmuls (on a GpSimd-memset tile) issue before any
  DMA-dependent work so the HAM clock gate's ~3.4 us busy window elapses
  during the input DMA and the chain runs at the warm clock.
- All 64 product packs on DVE (v3 showed GpSimd product offload causes
  SBUF contention that slows DVE 2-3x and starves the PE; each >3.4 us
  PE gap also re-cools the clock gate).
- DMA: the Sync engine issues descriptors serially (~0.6 us per
  instruction) after a ~7 us engine prologue, and all data DMAs share one
  hardware queue.  Rotations ship as ONE partition-major [D, 16*BLOC]
  dram tensor sliced into 5 column-range DMAs (2-8 KB contiguous runs per
  partition), interleaved with coefficient chunks in consumption order.
- Tail: per-bank eviction (Act bank0 / DVE bank1) with per-bank output
  DMAs, overlapping the final matmuls.

Sharding: data-parallel over batch B across the 8 cores (1024 rows each);
coefficient packs (derived from Sigmas/Centroids) are replicated.
"""

import os
import sys

sys.path.insert(0, "/opt/trn_rl_repo")

import numpy as np
import ml_dtypes

E, B, D = 128, 8192, 128
NCORES = 8
BLOC = B // NCORES          # 1024 batch rows per core
BT = 512                    # matmul free-dim tile (one PSUM bank)
NSLOT = 16                  # rotation slots: 0..7 then 8,16,...,64
ROTVALS = tuple(range(8)) + tuple(range(8, 65, 8))   # value of each slot
NWARM = 7                   # dummy matmuls to trip the HAM clock gate


# product groups: (name, in0 slot range [lo,hi), in1 slot).  Column i is
# rot[lo+i] * rot[in1], covering diagonal j = val(in1) - val(lo+i) with
# row rotation a = val(lo+i).  All on DVE, ordered by rotation-DMA batch.
GROUPS = [
    ("gA", 1, 3, 3),    # j = 2,1
    ("gB", 0, 1, 3),    # j = 3
    ("gC", 0, 4, 7),    # j = 7,6,5,4
    ("gD", 0, 1, 8),    # j = 8
    ("gE", 0, 8, 9),    # j = 16..9
    ("gF", 0, 8, 10),   # j = 24..17
    ("gG", 0, 8, 11),   # j = 32..25
    ("gH", 0, 8, 12),   # j = 40..33
    ("gI", 0, 8, 13),   # j = 48..41
    ("gJ", 0, 8, 14),   # j = 56..49
    ("gK", 0, 8, 15),   # j = 64..57
]

CHAIN = ["lin", "p0"] + [g[0] for g in GROUPS]
_GBY = {g[0]: g for g in GROUPS}
ORDER = []   # diagonal j per quad-pack position (positions 2..65)
AVAL = []    # row rotation a per quad-pack position
for _name in CHAIN[2:]:
    _, _lo, _hi, _s1 = _GBY[_name]
    for _i in range(_hi - _lo):
        ORDER.append(ROTVALS[_s1] - ROTVALS[_lo + _i])
        AVAL.append(ROTVALS[_lo + _i])

NPACK = 2 + len(ORDER)      # 66 emission positions
# coefficient chunks (pack counts); c0 carries linear+x^2 so the chain
# starts after a tiny transfer
CHUNKS = [2, 16, 24, 24]
CHUNK_OFF = [0]
for _n in CHUNKS:
    CHUNK_OFF.append(CHUNK_OFF[-1] + _n)


def _chunk_of(pos):
    for ci, off in enumerate(CHUNK_OFF[1:]):
        if pos < off:
            return ci, pos - CHUNK_OFF[ci]
    raise ValueError(pos)


bf16 = ml_dtypes.bfloat16

_STATE: dict = {}


def _build_module():
    import concourse.bacc as bacc
    import concourse.tile as tile
    import concourse.mybir as mybir
    from contextlib import ExitStack

    nc = bacc.Bacc("TRN2", target_bir_lowering=False, debug=False)

    # partition-major rotations: per partition 32 KB contiguous in DRAM
    xr_d = nc.dram_tensor("xrot", [D, NSLOT * BLOC], mybir.dt.bfloat16,
                          kind="ExternalInput")
    cw_d = nc.dram_tensor("cw", [D, NPACK * E], mybir.dt.bfloat16, kind="ExternalInput")
    tv_d = nc.dram_tensor("tv", [E, 1], mybir.dt.float32, kind="ExternalInput")
    out_d = nc.dram_tensor("out", [E, BLOC], mybir.dt.float32, kind="ExternalOutput")

    f32 = mybir.dt.float32
    b16 = mybir.dt.bfloat16
    Ident = mybir.ActivationFunctionType.Identity

    with tile.TileContext(nc) as tc, ExitStack() as ctx:
        const_pool = ctx.enter_context(tc.tile_pool(name="const", bufs=1))
        coef_pool = ctx.enter_context(tc.tile_pool(name="coef", bufs=1))
        p0_pool = ctx.enter_context(tc.tile_pool(name="p0", bufs=1))
        g_pool = ctx.enter_context(tc.tile_pool(name="g", bufs=3))
        psum_pool = ctx.enter_context(tc.tile_pool(name="acc", bufs=2, space="PSUM"))
        out_pool = ctx.enter_context(tc.tile_pool(name="outs", bufs=2))

        # PE warmup: dummy matmuls on a GpSimd-memset tile (GpSimd is the
        # earliest-ready engine and otherwise idle).  No DMA dependency, so
        # these run during the DMA prologue and warm the HAM clock gate.
        WU = const_pool.tile([D, BT], b16, tag="warm")
        nc.gpsimd.memset(WU[:, :], 0)
        PSW = psum_pool.tile([E, BT], f32, tag="psw", name="psw")
        for _ in range(NWARM):
            nc.tensor.matmul(PSW[:, :], WU[:, 0:E], WU[:, :],
                             start=True, stop=True, skip_group_check=True)

        ROTS = const_pool.tile([D, NSLOT * BLOC], b16, tag="rots")
        R3 = ROTS[:, :].rearrange("p (s b) -> p s b", s=NSLOT)
        TV = const_pool.tile([E, 1], f32, tag="tv")
        coef_tiles = [coef_pool.tile([D, n * E], b16, name=f"cw{ci}", tag=f"cw{ci}")
                      for ci, n in enumerate(CHUNKS)]

        def dma_rots(lo, hi):
            nc.sync.dma_start(ROTS[:, lo * BLOC:hi * BLOC],
                              xr_d.ap()[:, lo * BLOC:hi * BLOC])

        def dma_coef(ci):
            o0, o1 = CHUNK_OFF[ci] * E, CHUNK_OFF[ci + 1] * E
            nc.sync.dma_start(coef_tiles[ci][:], cw_d.ap()[:, o0:o1])

        # consumption order: slot0 + c0 unlock the linear/x^2 matmuls,
        # then rotation batches interleaved with coefficient chunks
        dma_rots(0, 1)
        dma_coef(0)
        dma_rots(1, 4)      # slots 1-3 -> gA, gB
        dma_coef(1)
        dma_rots(4, 8)      # slots 4-7 -> gC
        dma_rots(8, 12)     # slots 8,16,24,32 -> gD..gG
        dma_rots(12, 16)    # slots 40..64 -> gH..gK
        dma_coef(2)
        dma_coef(3)
        nc.sync.dma_start(TV[:], tv_d.ap())

        PS = psum_pool.tile([E, BLOC], f32, tag="ps", name="ps")

        def emit_pack(pos, rhs_ap_fn, start=False, stop=False):
            ci, cc = _chunk_of(pos)
            for bt in range(2):
                nc.tensor.matmul(PS[:, bt * BT:(bt + 1) * BT],
                                 coef_tiles[ci][:, cc * E:(cc + 1) * E],
                                 rhs_ap_fn(bt),
                                 start=start, stop=stop and bt == 1)

        # pos 0: linear term (rhs = x itself), starts the chain
        emit_pack(0, lambda bt: ROTS[:, bt * BT:(bt + 1) * BT], start=True)

        # pos 1: x^2 on the scalar engine
        PK0 = p0_pool.tile([D, BLOC], b16)
        nc.scalar.square(PK0[:, :], ROTS[:, 0:BLOC])
        emit_pack(1, lambda bt: PK0[:, bt * BT:(bt + 1) * BT])

        # product groups on DVE, matmuls chained right behind each group
        pos = 2
        for name, lo, hi, s1 in GROUPS:
            w = hi - lo
            PKg = g_pool.tile([D, w * BLOC], b16, name=f"PK{name}", tag="g")
            nc.vector.tensor_mul(
                PKg[:, :].rearrange("p (s b) -> p s b", s=w),
                R3[:, lo:hi, :],
                R3[:, s1:s1 + 1, :].broadcast_to((D, w, BLOC)),
            )
            for i in range(w):
                emit_pack(pos, lambda bt, _i=i, _t=PKg: _t[:, _i * BLOC + bt * BT:
                                                          _i * BLOC + (bt + 1) * BT],
                          stop=(pos == NPACK - 1))
                pos += 1

        # per-bank eviction + output DMA, overlapping the tail matmuls
        OT0 = out_pool.tile([E, BT], f32, tag="ot")
        OT1 = out_pool.tile([E, BT], f32, tag="ot")
        nc.scalar.activation(OT0[:, :], PS[:, 0:BT], Ident, bias=TV[:, 0:1])
        nc.sync.dma_start(out_d.ap()[:, 0:BT], OT0[:, :])
        nc.vector.tensor_scalar_add(OT1[:, :], PS[:, BT:BLOC], TV[:, 0:1])
        nc.sync.dma_start(out_d.ap()[:, BT:BLOC], OT1[:, :])

    nc.compile()
    return nc


def _host_precompute(Centroids: np.ndarray, Sigmas: np.ndarray):
    """Coefficient packs from the (replicated) small parameters."""
    Sinv = np.linalg.inv(Sigmas.astype(np.float64))
    A = 0.5 * (Sinv + np.swapaxes(Sinv, 1, 2))          # [E, D, D] symmetric
    c = Centroids[:, 0, :].astype(np.float64)           # [E, D]
    Ac = np.einsum("edk,ek->ed", A, c)

    cw = np.zeros((D, NPACK, E), np.float32)            # [row, emission pos, e]
    idx = np.arange(D)
    cw[:, 0, :] = (-2.0 * Ac.T)                         # linear term
    cw[:, 1, :] = A[:, idx, idx].T                      # x^2 diagonal
    for k, j in enumerate(ORDER):
        s = 2.0 if 1 <= j <= 63 else 1.0
        a = AVAL[k]
        cw[:, 2 + k, :] = s * A[:, (idx + a) % D, (idx + a + j) % D].T
    cw_host = np.ascontiguousarray(cw.reshape(D, NPACK * E)).astype(bf16)
    tv_host = np.ascontiguousarray(
        np.einsum("ed,ed->e", Ac, c).astype(np.float32)[:, None]
    )                                                                    # [E, 1]
    return cw_host, tv_host


def _get_nc():
    if "nc" not in _STATE:
        os.environ.setdefault("JAX_COMPILATION_CACHE_DIR", "/root/.jax_cache")
        _STATE["nc"] = _build_module()
    return _STATE["nc"]


def _make_in_maps(x, Centroids, Sigmas):
    cw_host, tv_host = _host_precompute(
        np.asarray(Centroids, np.float32), np.asarray(Sigmas, np.float32)
    )
    xT = np.ascontiguousarray(np.asarray(x, np.float32).T).astype(bf16)  # [D, B]
    in_maps = []
    for cidx in range(NCORES):
        xTs = np.ascontiguousarray(xT[:, cidx * BLOC:(cidx + 1) * BLOC])
        xrot = np.concatenate(
            [np.roll(xTs, -r, axis=0) for r in ROTVALS], axis=1)   # [D, 16*BLOC]
        in_maps.append({
            "xrot": np.ascontiguousarray(xrot),
            "cw": cw_host,
            "tv": tv_host,
        })
    return in_maps


def _run_device(in_maps, trace=False):
    from concourse import bass_utils

    nc = _get_nc()
    return bass_utils.run_bass_kernel_spmd(
        nc, in_maps, core_ids=list(range(NCORES)), trace=trace
    )


def kernel(x, Centroids, Sigmas):
    in_maps = _make_in_maps(x, Centroids, Sigmas)
    res = _run_device(in_maps)
    outT = np.concatenate([res.results[c]["out"] for c in range(NCORES)], axis=1)
    return np.ascontiguousarray(outT.T).astype(np.float32)
